# revision 1
# baseline (speedup 1.0000x reference)
"""GQA attention kernel for Trainium2, 8 NeuronCores.

Problem: x[2,2048,2048] @ Wq/Wk/Wv -> grouped-query attention (16 q heads,
4 kv groups, head_dim 128, causal) -> @ Wo + bo.

Sharding: (batch b in 0..1) x (kv group g in 0..3) -> 8 cores.
Each core computes the full attention for its (b, g): 4 query heads sharing
one kv head, then a row-parallel partial of the output projection
(ctx_g @ Wo[g*512:(g+1)*512, :]). Host sums the 4 group partials per batch
and adds the bias.

Device-side layout strategy (all matmuls in float32r, 1 cycle/row):
  - host ships xT = x[b].T so feature dim c is the SBUF partition dim
  - per query block of 512: qT[r] [d=128, i], kT [d=128, i] via lhsT=W, rhs=xT
  - scores computed transposed: sT[j, i] = kT_tile.T @ qT (j = key pos)
  - exp on ACT engine (scale=1/sqrt(128) folded in), causal mask via
    gpsimd affine_select on the diagonal tiles only
  - denominator: DVE accumulates exp tiles, gpsimd partition-reduce (axis C)
  - ctxT[d, i] accumulated in PSUM with lhsT = v natural [j, d]
  - v natural obtained from vT via PE transpose
  - out[i, :] = sum_r ctxnT_r.T @ Wo_rows, fused per block (A->B->C per block)
"""

import os

import ml_dtypes
import numpy as np

import concourse.bass as bass
from concourse import bacc
import concourse.bass_isa as bass_isa
import concourse.mybir as mybir
from concourse.bass_utils import run_bass_kernel_spmd
from concourse.masks import make_identity
from concourse.tile import TileContext

B, N, D = 2, 2048, 2048
G, REP, HD = 4, 4, 128
E = REP * HD  # 512 q-dims per group
P = 128
IB = 512  # i-block (query block) size
NBLK = N // IB  # 4
NCT = D // P  # 16 contraction tiles
NJT = N // P  # 16 key tiles
SCALE = 1.0 / float(np.sqrt(HD))

F32 = mybir.dt.float32
F32R = mybir.dt.float32r
BF16 = mybir.dt.bfloat16

_LAST_RESULT = None  # test.py reads exec_time_ns from here


def _r(ap):
    return ap.bitcast(F32R)


def build_bass():
    nc = bacc.Bacc()
    xT = nc.dram_tensor("xT", [D, N], F32R, kind="ExternalInput")
    wq = nc.dram_tensor("wq", [D, E], F32R, kind="ExternalInput")
    wk = nc.dram_tensor("wk", [D, HD], F32R, kind="ExternalInput")
    wv = nc.dram_tensor("wv", [D, HD], F32R, kind="ExternalInput")
    wo = nc.dram_tensor("wo", [E, D], BF16, kind="ExternalInput")
    out = nc.dram_tensor("out", [N, D], F32, kind="ExternalOutput")

    with TileContext(nc) as tc:
        build_tile_kernel(nc, tc, xT, wq, wk, wv, wo, out)
    nc.finalize()
    return nc


def build_tile_kernel(nc, tc, xT, wq, wk, wv, wo, out):
    import contextlib

    ctx = contextlib.ExitStack()
    with ctx:
        persist = ctx.enter_context(tc.tile_pool(name="persist", bufs=1))
        weights = ctx.enter_context(tc.tile_pool(name="weights", bufs=1))
        work = ctx.enter_context(tc.tile_pool(name="work", bufs=2))
        psum_mm = ctx.enter_context(
            tc.tile_pool(name="psum_mm", bufs=2, space="PSUM")
        )
        psum_ctx = ctx.enter_context(
            tc.tile_pool(name="psum_ctx", bufs=2, space="PSUM")
        )
        psum_aux = ctx.enter_context(
            tc.tile_pool(name="psum_aux", bufs=2, space="PSUM")
        )

        # ---- constants ----
        ident = persist.tile([P, P], F32)
        make_identity(nc, ident)

        # ---- weights in SBUF ----
        wq_sb = []
        for ct in range(NCT):
            t = weights.tile([P, E], F32R, name=f"wq{ct}", tag="wq", bufs=NCT)
            nc.sync.dma_start(out=t, in_=wq[ct * P : (ct + 1) * P, :])
            wq_sb.append(t)
        wk_sb = []
        wv_sb = []
        for ct in range(NCT):
            t = weights.tile([P, HD], F32R, name=f"wk{ct}", tag="wk", bufs=NCT)
            nc.sync.dma_start(out=t, in_=wk[ct * P : (ct + 1) * P, :])
            wk_sb.append(t)
            t = weights.tile([P, HD], F32R, name=f"wv{ct}", tag="wv", bufs=NCT)
            nc.sync.dma_start(out=t, in_=wv[ct * P : (ct + 1) * P, :])
            wv_sb.append(t)
        wo_sb = []
        for r in range(REP):
            t = weights.tile([P, D], BF16, name=f"wo{r}", tag="wo", bufs=REP)
            nc.sync.dma_start(out=t, in_=wo[r * P : (r + 1) * P, :])
            wo_sb.append(t)

        # persistent activations (full sequence)
        kT = persist.tile([P, N], BF16)  # [d, i]
        v_sb = [
            persist.tile([P, HD], BF16, name=f"v{jt}", tag="v", bufs=NJT)
            for jt in range(NJT)
        ]

        for ib in range(NBLK):
            isl = slice(ib * IB, (ib + 1) * IB)

            # ============ A: projections for this block ====================
            xt_b = []
            for ct in range(NCT):
                t = work.tile([P, IB], F32R, name=f"xt{ib}_{ct}", tag="xt", bufs=18)
                nc.sync.dma_start(out=t, in_=xT[ct * P : (ct + 1) * P, isl])
                xt_b.append(t)

            ps_k = psum_mm.tile([P, IB], F32, name=f"psk{ib}", tag="mm")
            for ct in range(NCT):
                nc.tensor.matmul(
                    ps_k,
                    lhsT=wk_sb[ct],
                    rhs=xt_b[ct],
                    start=(ct == 0),
                    stop=(ct == NCT - 1),
                )
            nc.scalar.copy(kT[:, isl], ps_k)

            ps_v = psum_mm.tile([P, IB], F32, name=f"psv{ib}", tag="mm")
            for ct in range(NCT):
                nc.tensor.matmul(
                    ps_v,
                    lhsT=wv_sb[ct],
                    rhs=xt_b[ct],
                    start=(ct == 0),
                    stop=(ct == NCT - 1),
                )
            vT_b = work.tile([P, IB], F32, name=f"vT{ib}", tag="vT", bufs=2)
            nc.scalar.copy(vT_b, ps_v)
            for sub in range(IB // P):
                jt = ib * (IB // P) + sub
                ps_t = psum_aux.tile([P, P], F32, name=f"pst{jt}", tag="aux")
                nc.tensor.transpose(
                    ps_t, vT_b[:, sub * P : (sub + 1) * P], ident
                )
                nc.scalar.copy(v_sb[jt], ps_t)

            qT_b = []
            for r in range(REP):
                ps_q = psum_mm.tile([P, IB], F32, name=f"psq{ib}_{r}", tag="mm")
                for ct in range(NCT):
                    nc.tensor.matmul(
                        ps_q,
                        lhsT=wq_sb[ct][:, r * P : (r + 1) * P],
                        rhs=xt_b[ct],
                        start=(ct == 0),
                        stop=(ct == NCT - 1),
                    )
                qt = work.tile([P, IB], BF16, name=f"qT{ib}_{r}", tag="qT", bufs=8)
                nc.scalar.copy(qt, ps_q)
                qT_b.append(qt)

            # ============ B: attention for this query block ================
            # jk outer / r inner: 4 live ctx accumulation groups (one per
            # head), scores + ctx matmuls for one key tile share kT/v tiles.
            njt = (ib + 1) * (IB // P)  # causal: key tiles 0..njt-1
            ps_cs = [
                psum_ctx.tile([P, IB], F32, name=f"psc{ib}_{r}", tag=f"ctx{r}", bufs=1)
                for r in range(REP)
            ]
            accs = [
                work.tile([P, IB], F32, name=f"acc{ib}_{r}", tag=f"acc{r}", bufs=1)
                for r in range(REP)
            ]
            for jk in range(njt):
                m = jk - ib * (IB // P)
                for r in range(REP):
                    ps_s = psum_mm.tile(
                        [P, IB], F32, name=f"pss{ib}_{r}_{jk}", tag="mm"
                    )
                    nc.tensor.matmul(
                        ps_s,
                        lhsT=kT[:, jk * P : (jk + 1) * P],
                        rhs=qT_b[r],
                        start=True,
                        stop=True,
                    )
                    ex = work.tile(
                        [P, IB], BF16, name=f"ex{ib}_{r}_{jk}", tag="ex", bufs=4
                    )
                    nc.scalar.activation(
                        ex, ps_s, mybir.ActivationFunctionType.Exp, scale=SCALE
                    )
                    if m >= 0:
                        # diagonal tile: keep where i - j - 128*m >= 0, else 0
                        nc.gpsimd.affine_select(
                            out=ex,
                            in_=ex,
                            compare_op=mybir.AluOpType.is_ge,
                            fill=0.0,
                            base=-(P * m),
                            pattern=[[1, IB]],
                            channel_multiplier=-1,
                        )
                    nc.tensor.matmul(
                        ps_cs[r],
                        lhsT=v_sb[jk],
                        rhs=ex,
                        start=(jk == 0),
                        stop=(jk == njt - 1),
                    )
                    if jk == 0:
                        nc.vector.tensor_copy(accs[r], ex)
                    else:
                        nc.vector.tensor_add(accs[r], accs[r], ex)
            ctxn_b = []
            for r in range(REP):
                sred = work.tile([P, IB], F32, name=f"sr{ib}_{r}", tag="sred", bufs=2)
                nc.gpsimd.partition_all_reduce(
                    sred, accs[r], channels=P, reduce_op=bass_isa.ReduceOp.add
                )
                rec1 = work.tile([1, IB], F32, name=f"r1{ib}_{r}", tag="rec1", bufs=2)
                nc.vector.reciprocal(rec1, sred[0:1, :])
                recb = work.tile([P, IB], F32, name=f"rb{ib}_{r}", tag="recb", bufs=2)
                nc.gpsimd.partition_broadcast(recb, rec1)
                cn = work.tile([P, IB], BF16, name=f"cn{ib}_{r}", tag="ctxn", bufs=8)
                nc.vector.tensor_mul(cn, ps_cs[r], recb)
                ctxn_b.append(cn)

            # ============ C: output projection for this block ==============
            for sub in range(IB // P):
                it = ib * (IB // P) + sub
                ssl = slice(sub * P, (sub + 1) * P)
                o_sb = work.tile([P, D], F32, name=f"osb{it}", tag="osb", bufs=2)
                for ot in range(D // IB):
                    ps_o = psum_mm.tile([P, IB], F32, name=f"pso{it}_{ot}", tag="mm")
                    for r in range(REP):
                        nc.tensor.matmul(
                            ps_o,
                            lhsT=ctxn_b[r][:, ssl],
                            rhs=wo_sb[r][:, ot * IB : (ot + 1) * IB],
                            start=(r == 0),
                            stop=(r == REP - 1),
                        )
                    nc.vector.tensor_copy(o_sb[:, ot * IB : (ot + 1) * IB], ps_o)
                nc.sync.dma_start(out=out[it * P : (it + 1) * P, :], in_=o_sb)


_NC_CACHE = None


def kernel(x, Wq, Wk, Wv, Wo, bo):
    global _LAST_RESULT, _NC_CACHE
    x = np.asarray(x, dtype=np.float32)
    Wq = np.asarray(Wq, dtype=np.float32)
    Wk = np.asarray(Wk, dtype=np.float32)
    Wv = np.asarray(Wv, dtype=np.float32)
    Wo = np.asarray(Wo, dtype=np.float32)
    bo = np.asarray(bo, dtype=np.float32)

    if _NC_CACHE is None:
        _NC_CACHE = build_bass()
    nc = _NC_CACHE

    in_maps = []
    for core in range(8):
        b, g = core // G, core % G
        in_maps.append(
            {
                "xT": np.ascontiguousarray(x[b].T),
                "wq": np.ascontiguousarray(Wq[:, g * E : (g + 1) * E]),
                "wk": np.ascontiguousarray(Wk[:, g * HD : (g + 1) * HD]),
                "wv": np.ascontiguousarray(Wv[:, g * HD : (g + 1) * HD]),
                "wo": np.ascontiguousarray(Wo[g * E : (g + 1) * E, :]).astype(ml_dtypes.bfloat16),
            }
        )
    res = run_bass_kernel_spmd(
        nc,
        in_maps,
        list(range(8)),
        trace=bool(os.environ.get("BASS_TRACE")),
    )
    _LAST_RESULT = res
    partials = np.stack([r["out"] for r in res.results])  # [8, N, D]
    full = partials.reshape(B, G, N, D).sum(axis=1) + bo[None, None, :]
    return full.astype(np.float32)



# revision 6
# speedup vs baseline: 1.2088x; 1.2088x over previous
"""GQA attention kernel for Trainium2, 8 NeuronCores.

Problem: x[2,2048,2048] @ Wq/Wk/Wv -> grouped-query attention (16 q heads,
4 kv groups, head_dim 128, causal) -> @ Wo + bo.

Sharding: (batch b in 0..1) x (kv group g in 0..3) -> 8 cores.
Each core computes the full attention for its (b, g): 4 query heads sharing
one kv head, then a row-parallel partial of the output projection
(ctx_g @ Wo[g*512:(g+1)*512, :]). Host sums the 4 group partials per batch
and adds the bias.

v2 changes vs baseline (461us):
  - softmax denominator summed on the PE (lhsT=[128,4] one-hot-column ones)
    into a single [4,512] PSUM bank, replacing 160 DVE adds + gpsimd
    partition_all_reduce.
  - reciprocal via DVE reciprocal_approx_fast on [4,512] (was 3.3us/row
    serial InstReciprocal).
  - reciprocal broadcast via PE matmul (lhsT=[4,128] one-hot-row ones),
    replacing gpsimd partition_broadcast.
  - causal diagonal tiles compute only the live column range i >= 128*m;
    affine_select only on the [128,128] triangular strip.
Goal: PE never idles > ~3.4us (stays at 2.4GHz), no DVE/gpsimd critical path.
"""

import os

import ml_dtypes
import numpy as np

import concourse.bass as bass
from concourse import bacc
import concourse.bass_isa as bass_isa
import concourse.mybir as mybir
from concourse.bass_utils import run_bass_kernel_spmd
from concourse.masks import make_identity
from concourse.tile import TileContext

B, N, D = 2, 2048, 2048
G, REP, HD = 4, 4, 128
E = REP * HD  # 512 q-dims per group
P = 128
IB = 512  # i-block (query block) size
NBLK = N // IB  # 4
NCT = D // P  # 16 contraction tiles
NJT = N // P  # 16 key tiles
SCALE = 1.0 / float(np.sqrt(HD))

F32 = mybir.dt.float32
F32R = mybir.dt.float32r
BF16 = mybir.dt.bfloat16

_LAST_RESULT = None  # test.py reads exec_time_ns from here


def _r(ap):
    return ap.bitcast(F32R)


def build_bass():
    nc = bacc.Bacc()
    xT = nc.dram_tensor("xT", [D, N], F32R, kind="ExternalInput")
    wq = nc.dram_tensor("wq", [D, E], F32R, kind="ExternalInput")
    wk = nc.dram_tensor("wk", [D, HD], F32R, kind="ExternalInput")
    wv = nc.dram_tensor("wv", [D, HD], F32R, kind="ExternalInput")
    wo = nc.dram_tensor("wo", [E, D], BF16, kind="ExternalInput")
    out = nc.dram_tensor("out", [N, D], F32, kind="ExternalOutput")

    with TileContext(nc) as tc:
        build_tile_kernel(nc, tc, xT, wq, wk, wv, wo, out)
    nc.finalize()
    return nc


def build_tile_kernel(nc, tc, xT, wq, wk, wv, wo, out):
    import contextlib

    ctx = contextlib.ExitStack()
    with ctx:
        persist = ctx.enter_context(tc.tile_pool(name="persist", bufs=1))
        weights = ctx.enter_context(tc.tile_pool(name="weights", bufs=1))
        work = ctx.enter_context(tc.tile_pool(name="work", bufs=2))
        psum_mm = ctx.enter_context(
            tc.tile_pool(name="psum_mm", bufs=2, space="PSUM")
        )
        psum_ctx = ctx.enter_context(
            tc.tile_pool(name="psum_ctx", bufs=2, space="PSUM")
        )
        psum_den = ctx.enter_context(
            tc.tile_pool(name="psum_den", bufs=1, space="PSUM")
        )
        psum_aux = ctx.enter_context(
            tc.tile_pool(name="psum_aux", bufs=1, space="PSUM")
        )

        # ---- constants ----
        ident = persist.tile([P, P], F32)
        make_identity(nc, ident)
        # sel_ones[r]: [128,4] bf16, column r all ones (den matmul lhsT)
        sel_ones = []
        for r in range(REP):
            t = persist.tile([P, REP], BF16, name=f"selo{r}", tag="selo", bufs=REP)
            nc.vector.memset(t, 0.0)
            nc.vector.memset(t[:, r : r + 1], 1.0)
            sel_ones.append(t)
        # sel4[r]: [4,128] bf16, row r all ones (reciprocal broadcast lhsT).
        # Partition-sliced memset fails BIR verification, so carve the row
        # out of an all-ones tile with affine_select on the channel index.
        sel4 = []
        for r in range(REP):
            t = persist.tile([REP, P], BF16, name=f"sel4{r}", tag="sel4", bufs=REP)
            nc.vector.memset(t, 1.0)
            nc.gpsimd.affine_select(
                out=t,
                in_=t,
                compare_op=mybir.AluOpType.is_equal,
                fill=0.0,
                base=-r,
                pattern=[[0, P]],
                channel_multiplier=1,
            )
            sel4.append(t)

        # ---- weights in SBUF ----
        wq_sb = []
        for ct in range(NCT):
            t = weights.tile([P, E], F32R, name=f"wq{ct}", tag="wq", bufs=NCT)
            nc.sync.dma_start(out=t, in_=wq[ct * P : (ct + 1) * P, :])
            wq_sb.append(t)
        wk_sb = []
        wv_sb = []
        for ct in range(NCT):
            t = weights.tile([P, HD], F32R, name=f"wk{ct}", tag="wk", bufs=NCT)
            nc.sync.dma_start(out=t, in_=wk[ct * P : (ct + 1) * P, :])
            wk_sb.append(t)
            t = weights.tile([P, HD], F32R, name=f"wv{ct}", tag="wv", bufs=NCT)
            nc.sync.dma_start(out=t, in_=wv[ct * P : (ct + 1) * P, :])
            wv_sb.append(t)
        wo_sb = []
        for r in range(REP):
            t = weights.tile([P, D], BF16, name=f"wo{r}", tag="wo", bufs=REP)
            nc.sync.dma_start(out=t, in_=wo[r * P : (r + 1) * P, :])
            wo_sb.append(t)

        # persistent activations (full sequence)
        kT = persist.tile([P, N], BF16)  # [d, i]
        v_sb = [
            persist.tile([P, HD], BF16, name=f"v{jt}", tag="v", bufs=NJT)
            for jt in range(NJT)
        ]

        for ib in range(NBLK):
            isl = slice(ib * IB, (ib + 1) * IB)

            # ============ A: projections for this block ====================
            xt_b = []
            for ct in range(NCT):
                t = work.tile([P, IB], F32R, name=f"xt{ib}_{ct}", tag="xt", bufs=18)
                nc.sync.dma_start(out=t, in_=xT[ct * P : (ct + 1) * P, isl])
                xt_b.append(t)

            ps_k = psum_mm.tile([P, IB], F32, name=f"psk{ib}", tag="mm")
            for ct in range(NCT):
                nc.tensor.matmul(
                    ps_k,
                    lhsT=wk_sb[ct],
                    rhs=xt_b[ct],
                    start=(ct == 0),
                    stop=(ct == NCT - 1),
                )
            nc.scalar.copy(kT[:, isl], ps_k)

            ps_v = psum_mm.tile([P, IB], F32, name=f"psv{ib}", tag="mm")
            for ct in range(NCT):
                nc.tensor.matmul(
                    ps_v,
                    lhsT=wv_sb[ct],
                    rhs=xt_b[ct],
                    start=(ct == 0),
                    stop=(ct == NCT - 1),
                )
            vT_b = work.tile([P, IB], F32, name=f"vT{ib}", tag="vT", bufs=2)
            nc.scalar.copy(vT_b, ps_v)
            for sub in range(IB // P):
                jt = ib * (IB // P) + sub
                ps_t = psum_aux.tile([P, IB], F32, name=f"pst{jt}", tag="aux")
                nc.tensor.transpose(
                    ps_t[:, 0:P], vT_b[:, sub * P : (sub + 1) * P], ident
                )
                nc.scalar.copy(v_sb[jt], ps_t[:, 0:P])

            qT_b = []
            for r in range(REP):
                ps_q = psum_mm.tile([P, IB], F32, name=f"psq{ib}_{r}", tag="mm")
                for ct in range(NCT):
                    nc.tensor.matmul(
                        ps_q,
                        lhsT=wq_sb[ct][:, r * P : (r + 1) * P],
                        rhs=xt_b[ct],
                        start=(ct == 0),
                        stop=(ct == NCT - 1),
                    )
                qt = work.tile([P, IB], BF16, name=f"qT{ib}_{r}", tag="qT", bufs=8)
                nc.scalar.copy(qt, ps_q)
                qT_b.append(qt)

            # ============ B: attention for this query block ================
            # jk outer / r inner; ctx accumulated per head in PSUM, softmax
            # denominator accumulated on the PE into den4 [4, 512].
            njt = (ib + 1) * (IB // P)  # causal: key tiles 0..njt-1
            ps_cs = [
                psum_ctx.tile([P, IB], F32, name=f"psc{ib}_{r}", tag=f"ctx{r}", bufs=1)
                for r in range(REP)
            ]
            den4 = psum_den.tile([REP, IB], F32, name=f"den{ib}", tag="den", bufs=1)
            for jk in range(njt):
                m = jk - ib * (IB // P)
                i0 = max(m, 0) * P  # live columns: i >= 128*m on diagonal
                for r in range(REP):
                    ps_s = psum_mm.tile(
                        [P, IB], F32, name=f"pss{ib}_{r}_{jk}", tag="mm"
                    )
                    nc.tensor.matmul(
                        ps_s[:, i0:],
                        lhsT=kT[:, jk * P : (jk + 1) * P],
                        rhs=qT_b[r][:, i0:],
                        start=True,
                        stop=True,
                    )
                    ex = work.tile(
                        [P, IB], BF16, name=f"ex{ib}_{r}_{jk}", tag="ex", bufs=6
                    )
                    nc.scalar.activation(
                        ex[:, i0:],
                        ps_s[:, i0:],
                        mybir.ActivationFunctionType.Exp,
                        scale=SCALE,
                    )
                    if m >= 0:
                        # triangular strip: keep where (i - i0) - j >= 0
                        nc.gpsimd.affine_select(
                            out=ex[:, i0 : i0 + P],
                            in_=ex[:, i0 : i0 + P],
                            compare_op=mybir.AluOpType.is_ge,
                            fill=0.0,
                            base=0,
                            pattern=[[1, P]],
                            channel_multiplier=-1,
                        )
                    nc.tensor.matmul(
                        den4[:, i0:],
                        lhsT=sel_ones[r],
                        rhs=ex[:, i0:],
                        start=(jk == 0 and r == 0),
                        stop=(jk == njt - 1 and r == REP - 1),
                        skip_group_check=True,
                    )
                    nc.tensor.matmul(
                        ps_cs[r][:, i0:],
                        lhsT=v_sb[jk],
                        rhs=ex[:, i0:],
                        start=(jk == 0),
                        stop=(jk == njt - 1),
                        skip_group_check=True,
                    )

            # ============ C: normalize + output projection =================
            rec4 = work.tile([REP, IB], F32, name=f"rec4{ib}", tag="rec4", bufs=2)
            nc.vector.reciprocal_approx_fast(out=rec4, in_=den4)
            rec4b = work.tile([REP, IB], BF16, name=f"rec4b{ib}", tag="rec4b", bufs=2)
            nc.vector.tensor_copy(rec4b, rec4)
            ctxn_b = []
            for r in range(REP):
                ps_rb = psum_aux.tile([P, IB], F32, name=f"psrb{ib}_{r}", tag="aux")
                nc.tensor.matmul(
                    ps_rb,
                    lhsT=sel4[r],
                    rhs=rec4b,
                    start=True,
                    stop=True,
                )
                rb_sb = work.tile([P, IB], BF16, name=f"rb{ib}_{r}", tag="rb", bufs=2)
                nc.scalar.copy(rb_sb, ps_rb)
                cn = work.tile([P, IB], BF16, name=f"cn{ib}_{r}", tag="ctxn", bufs=8)
                nc.vector.tensor_mul(cn, ps_cs[r], rb_sb)
                ctxn_b.append(cn)

            for sub in range(IB // P):
                it = ib * (IB // P) + sub
                ssl = slice(sub * P, (sub + 1) * P)
                o_sb = work.tile([P, D], F32, name=f"osb{it}", tag="osb", bufs=2)
                for ot in range(D // IB):
                    ps_o = psum_mm.tile([P, IB], F32, name=f"pso{it}_{ot}", tag="mm")
                    for r in range(REP):
                        nc.tensor.matmul(
                            ps_o,
                            lhsT=ctxn_b[r][:, ssl],
                            rhs=wo_sb[r][:, ot * IB : (ot + 1) * IB],
                            start=(r == 0),
                            stop=(r == REP - 1),
                        )
                    nc.vector.tensor_copy(o_sb[:, ot * IB : (ot + 1) * IB], ps_o)
                nc.sync.dma_start(out=out[it * P : (it + 1) * P, :], in_=o_sb)


_NC_CACHE = None


def kernel(x, Wq, Wk, Wv, Wo, bo):
    global _LAST_RESULT, _NC_CACHE
    x = np.asarray(x, dtype=np.float32)
    Wq = np.asarray(Wq, dtype=np.float32)
    Wk = np.asarray(Wk, dtype=np.float32)
    Wv = np.asarray(Wv, dtype=np.float32)
    Wo = np.asarray(Wo, dtype=np.float32)
    bo = np.asarray(bo, dtype=np.float32)

    if _NC_CACHE is None:
        _NC_CACHE = build_bass()
    nc = _NC_CACHE

    in_maps = []
    for core in range(8):
        b, g = core // G, core % G
        in_maps.append(
            {
                "xT": np.ascontiguousarray(x[b].T),
                "wq": np.ascontiguousarray(Wq[:, g * E : (g + 1) * E]),
                "wk": np.ascontiguousarray(Wk[:, g * HD : (g + 1) * HD]),
                "wv": np.ascontiguousarray(Wv[:, g * HD : (g + 1) * HD]),
                "wo": np.ascontiguousarray(Wo[g * E : (g + 1) * E, :]).astype(ml_dtypes.bfloat16),
            }
        )
    res = run_bass_kernel_spmd(
        nc,
        in_maps,
        list(range(8)),
        trace=bool(os.environ.get("BASS_TRACE")),
    )
    _LAST_RESULT = res
    partials = np.stack([r["out"] for r in res.results])  # [8, N, D]
    full = partials.reshape(B, G, N, D).sum(axis=1) + bo[None, None, :]
    return full.astype(np.float32)
